# revision 1
# baseline (speedup 1.0000x reference)
import math
import sys

import numpy as np

for _p in ("/opt/trn_rl_repo",):
    if _p not in sys.path:
        sys.path.insert(0, _p)

from concourse import bass, mybir
from concourse.tile import TileContext
from concourse.bass_utils import run_bass_kernel_spmd
from concourse.masks import make_upper_triangular

N = 4096
H = 384
W = 384
FOCAL = 0.5 * W / math.tan(0.5 * math.pi / 2.0)
CX, CY = W / 2.0, H / 2.0
CLIP_Z = 0.01
BLUR = 0.3
ALPHA_MIN = 1.0 / 255.0
NCORES = 8
PTILE = 512
GBLK = 128

f32 = mybir.dt.float32
AF = mybir.ActivationFunctionType
OP = mybir.AluOpType


def _preprocess(xyz, scaling, opacity, rotation, features_dc):
    """Project gaussians (float64 on host), depth-sort, return per-gaussian
    screen params in front-to-back order."""
    xyz = xyz.astype(np.float64)
    x, y = xyz[:, 0], xyz[:, 1]
    z = xyz[:, 2] + 8.0
    zs = np.where(z > CLIP_Z, z, 1.0)

    scales = np.exp(scaling.astype(np.float64))
    q = rotation.astype(np.float64)
    q = q / np.linalg.norm(q, axis=-1, keepdims=True)
    w_, qx, qy, qz = q[:, 0], q[:, 1], q[:, 2], q[:, 3]
    R = np.empty((N, 3, 3), np.float64)
    R[:, 0, 0] = 1 - 2 * (qy * qy + qz * qz)
    R[:, 0, 1] = 2 * (qx * qy - w_ * qz)
    R[:, 0, 2] = 2 * (qx * qz + w_ * qy)
    R[:, 1, 0] = 2 * (qx * qy + w_ * qz)
    R[:, 1, 1] = 1 - 2 * (qx * qx + qz * qz)
    R[:, 1, 2] = 2 * (qy * qz - w_ * qx)
    R[:, 2, 0] = 2 * (qx * qz - w_ * qy)
    R[:, 2, 1] = 2 * (qy * qz + w_ * qx)
    R[:, 2, 2] = 1 - 2 * (qx * qx + qy * qy)
    M = R * scales[:, None, :]
    cov3d = np.einsum('nij,nkj->nik', M, M)

    tan_f = 0.5 * W / FOCAL
    tx = zs * np.clip(x / zs, -1.3 * tan_f, 1.3 * tan_f)
    ty = zs * np.clip(y / zs, -1.3 * tan_f, 1.3 * tan_f)
    rz, rz2 = 1.0 / zs, 1.0 / (zs * zs)
    J = np.zeros((N, 2, 3), np.float64)
    J[:, 0, 0] = FOCAL * rz
    J[:, 0, 2] = -FOCAL * tx * rz2
    J[:, 1, 1] = FOCAL * rz
    J[:, 1, 2] = -FOCAL * ty * rz2
    cov2d = np.einsum('nij,njk,nlk->nil', J, cov3d, J)
    c00 = cov2d[:, 0, 0] + BLUR
    c01 = cov2d[:, 0, 1]
    c11 = cov2d[:, 1, 1] + BLUR
    det = c00 * c11 - c01 * c01
    valid = (z > CLIP_Z) & (det > 0.0)
    det_s = np.where(valid, det, 1.0)
    conic = np.stack([c11, -c01, c00], -1) / det_s[:, None]

    cx = FOCAL * x * rz + CX
    cy = FOCAL * y * rz + CY
    rgbs = 1.0 / (1.0 + np.exp(-features_dc[:, 0, :].astype(np.float64)))
    opac = 1.0 / (1.0 + np.exp(-opacity[:, 0].astype(np.float64))) * valid

    # conservative footprint radius: alpha >= ALPHA_MIN only possible within it
    lam_max = 0.5 * (c00 + c11) + 0.5 * np.sqrt((c00 - c11) ** 2 + 4 * c01 * c01)
    t_sig = np.log(np.maximum(opac, 1e-12) / ALPHA_MIN) + 0.02
    r = np.where(valid & (t_sig > 0), np.sqrt(2.0 * np.maximum(t_sig, 0) * lam_max) + 1.0, 0.0)

    order = np.argsort(np.where(valid, z, np.inf), kind='stable')
    return (conic[order], cx[order], cy[order], rgbs[order], opac[order],
            valid[order], r[order])


def _legalize_waits(nc):
    """The walrus codegen for compute-engine instruction structs accepts only
    one embedded sync wait. Move surplus waits onto same-engine NoOps placed
    immediately before the instruction."""
    skip = {"NoOp", "EventSemaphore", "Halt"}
    nid = [0]
    for blk in nc.main_func.blocks:
        out = []
        for inst in blk.instructions:
            si = getattr(inst, "sync_info", None)
            op = type(inst).__name__
            if (si is not None and si.on_wait and len(si.on_wait) > 1
                    and not any(s in op for s in skip)):
                waits = list(si.on_wait)
                for w in waits[:-1]:
                    nid[0] += 1
                    nop = mybir.InstNoOp(
                        name=f"{inst.name}-lw{nid[0]}", engine=inst.engine,
                        ins=[], outs=[],
                        sync_info=mybir.SyncInfo(on_wait=[w], on_update=[]))
                    out.append(nop)
                si.on_wait = [waits[-1]]
            out.append(inst)
        blk.instructions[:] = out


def kernel(xyz, scaling, opacity, rotation, features_dc):
    conic, cx, cy, rgbs, opac, valid, r = _preprocess(
        xyz, scaling, opacity, rotation, features_dc)

    live = valid & (opac > ALPHA_MIN) & (r > 0)
    out_img = np.ones((1, 3, H, W), np.float32)
    if not live.any():
        return out_img

    x0 = int(np.clip(np.floor((cx - r)[live].min()), 0, W - 1))
    x1 = int(np.clip(np.ceil((cx + r)[live].max()), 0, W - 1))
    y0 = int(np.clip(np.floor((cy - r)[live].min()), 0, H - 1))
    y1 = int(np.clip(np.ceil((cy + r)[live].max()), 0, H - 1))
    W_roi = x1 - x0 + 1
    H_roi = y1 - y0 + 1
    H_roi = ((H_roi + NCORES - 1) // NCORES) * NCORES
    if y0 + H_roi > H:
        y0 = H - H_roi
    rows_pc = H_roi // NCORES
    P_core = rows_pc * W_roi
    P_pad = ((P_core + PTILE - 1) // PTILE) * PTILE
    T = P_pad // PTILE

    xcen = x0 + (W_roi - 1) / 2.0
    ycen = y0 + (H_roi - 1) / 2.0
    cxr, cyr = cx - xcen, cy - ycen

    # per-core gaussian selection (front-to-back order preserved)
    sels = []
    for c in range(NCORES):
        ylo = y0 + c * rows_pc - ycen
        yhi = ylo + rows_pc - 1
        sels.append(np.nonzero(live & (cyr + r >= ylo) & (cyr - r <= yhi))[0])
    NB = max(1, max((len(s) + GBLK - 1) // GBLK for s in sels))
    NG = NB * GBLK

    in_maps = []
    for c in range(NCORES):
        s = sels[c]
        n = len(s)
        a6 = np.zeros((6, NG), np.float64)
        rgbt = np.zeros((GBLK, NB * 3), np.float32)
        logop = np.full((GBLK, NB), -1e4, np.float32)
        if n:
            c0, c1, c2 = conic[s, 0], conic[s, 1], conic[s, 2]
            gx, gy = cxr[s], cyr[s]
            a6[0, :n] = 0.5 * c0
            a6[1, :n] = 0.5 * c2
            a6[2, :n] = c1
            a6[3, :n] = -(c0 * gx + c1 * gy)
            a6[4, :n] = -(c2 * gy + c1 * gx)
            a6[5, :n] = 0.5 * (c0 * gx * gx + c2 * gy * gy) + c1 * gx * gy
            idx = np.arange(n)
            rgbt[idx % GBLK, (idx // GBLK) * 3 + 0] = rgbs[s, 0]
            rgbt[idx % GBLK, (idx // GBLK) * 3 + 1] = rgbs[s, 1]
            rgbt[idx % GBLK, (idx // GBLK) * 3 + 2] = rgbs[s, 2]
            logop[idx % GBLK, idx // GBLK] = np.log(opac[s])

        yy, xx = np.meshgrid(np.arange(rows_pc) + y0 + c * rows_pc,
                             np.arange(W_roi) + x0, indexing='ij')
        fx = np.full(P_pad, 1e4, np.float64)
        fy = np.full(P_pad, 1e4, np.float64)
        fx[:P_core] = (xx - xcen).ravel()
        fy[:P_core] = (yy - ycen).ravel()
        feat = np.stack([fx * fx, fy * fy, fx * fy, fx, fy,
                         np.ones(P_pad)], 0)
        # pack everything into one [128, C] blob: single DMA, single wait
        C_lm, C_on, C_rgb, C_lop, C_a6, C_ft = 0, GBLK, 2 * GBLK, 2 * GBLK + 3 * NB, \
            2 * GBLK + 3 * NB + NB, 2 * GBLK + 4 * NB + NG
        C = C_ft + P_pad
        blob = np.zeros((GBLK, C), np.float32)
        blob[:, C_lm:C_lm + GBLK] = np.triu(np.ones((GBLK, GBLK), np.float32), 1)
        blob[:, C_on:C_on + GBLK] = 1.0
        blob[:, C_rgb:C_rgb + 3 * NB] = rgbt
        blob[:, C_lop:C_lop + NB] = logop
        blob[:6, C_a6:C_a6 + NG] = a6.astype(np.float32)
        blob[:6, C_ft:C_ft + P_pad] = feat.astype(np.float32)
        in_maps.append({"blob": blob})

    nc = bass.Bass()
    blob_d = nc.declare_dram_parameter("blob", [GBLK, C], f32, isOutput=False)
    out_d = nc.declare_dram_parameter("out", [3, P_pad], f32, isOutput=True)

    with TileContext(nc) as tc:
        with tc.tile_pool(name="const", bufs=1) as cp, \
             tc.tile_pool(name="work", bufs=3) as wp, \
             tc.tile_pool(name="ps", bufs=2, space="PSUM") as pp, \
             tc.tile_pool(name="pimg", bufs=2, space="PSUM") as ip:
            blob_sb = cp.tile([GBLK, C], f32)
            nc.sync.dma_start(out=blob_sb[:], in_=blob_d[:])
            # warm-ups: absorb the blob DMA-HW wait once per consuming engine
            scr = cp.tile([1, 4], f32)
            scrv = cp.tile([1, 4], f32)
            pscr = pp.tile([1, 1], f32, tag="bsum")
            nc.scalar.activation(out=scr[0:1, 0:1], in_=blob_sb[0:1, 0:1],
                                 func=AF.Copy)
            nc.vector.tensor_scalar_add(scrv[0:1, 0:1], blob_sb[0:1, 0:1], 0.0)
            nc.tensor.matmul(pscr[:], blob_sb[0:1, 0:1], blob_sb[0:1, 0:1],
                             start=True, stop=True)

            for t in range(T):
                carry = wp.tile([1, PTILE], f32, tag="carry")
                nc.vector.memset(carry[:], 0.0)
                pimg = ip.tile([3, PTILE], f32, tag="img")
                for b in range(NB):
                    psig = pp.tile([GBLK, PTILE], f32, tag="sig")
                    nc.tensor.matmul(psig[:],
                                     blob_sb[0:6, C_a6 + b * GBLK:C_a6 + (b + 1) * GBLK],
                                     blob_sb[0:6, C_ft + t * PTILE:C_ft + (t + 1) * PTILE],
                                     start=True, stop=True)
                    # 1-elem splitter: absorbs the PE wait so the real act
                    # carries at most one sync wait (ACT ISA slot limit)
                    nc.scalar.activation(out=scr[0:1, 0:1], in_=psig[0:1, 0:1],
                                         func=AF.Copy)
                    araw = wp.tile([GBLK, PTILE], f32, tag="araw")
                    nc.scalar.activation(out=araw[:], in_=psig[:], func=AF.Exp,
                                         bias=blob_sb[:, C_lop + b:C_lop + b + 1],
                                         scale=-1.0)
                    m1 = wp.tile([GBLK, PTILE], f32, tag="m1")
                    nc.vector.tensor_scalar(m1[:], araw[:], ALPHA_MIN, None, OP.is_ge)
                    u = wp.tile([GBLK, PTILE], f32, tag="u")
                    nc.vector.tensor_tensor(u[:], araw[:], m1[:], OP.mult)
                    l1ma = wp.tile([GBLK, PTILE], f32, tag="l1ma")
                    nc.scalar.activation(out=l1ma[:], in_=u[:], func=AF.Ln,
                                         bias=blob_sb[:, C_on:C_on + 1], scale=-1.0)
                    pcum = pp.tile([GBLK, PTILE], f32, tag="cum")
                    nc.tensor.matmul(pcum[:], blob_sb[:, C_lm:C_lm + GBLK],
                                     l1ma[:], start=True, stop=False)
                    nc.tensor.matmul(pcum[:], blob_sb[0:1, C_on:C_on + GBLK],
                                     carry[:], start=False, stop=True)
                    nc.scalar.activation(out=scr[0:1, 1:2], in_=pcum[0:1, 0:1],
                                         func=AF.Copy)
                    tpre = wp.tile([GBLK, PTILE], f32, tag="tpre")
                    nc.scalar.activation(out=tpre[:], in_=pcum[:], func=AF.Exp,
                                         bias=blob_sb[:, C_lm:C_lm + 1])
                    pbsum = pp.tile([1, PTILE], f32, tag="bsum")
                    nc.tensor.matmul(pbsum[:], blob_sb[:, C_on:C_on + 1], l1ma[:],
                                     start=True, stop=True)
                    ncarry = wp.tile([1, PTILE], f32, tag="carry")
                    nc.vector.tensor_tensor(ncarry[:], pbsum[:], carry[:], OP.add)
                    carry = ncarry
                    w_t = wp.tile([GBLK, PTILE], f32, tag="w")
                    nc.vector.tensor_tensor(w_t[:], tpre[:], u[:], OP.mult)
                    nc.tensor.matmul(pimg[:],
                                     blob_sb[:, C_rgb + b * 3:C_rgb + (b + 1) * 3],
                                     w_t[:], start=(b == 0), stop=False)
                tfin = wp.tile([1, PTILE], f32, tag="tfin")
                nc.scalar.activation(out=tfin[:], in_=carry[:], func=AF.Exp,
                                     bias=blob_sb[0:1, C_lm:C_lm + 1])
                nc.tensor.matmul(pimg[:], blob_sb[0:1, C_on:C_on + 3], tfin[:],
                                 start=False, stop=True)
                outt = wp.tile([3, PTILE], f32, tag="outt")
                nc.vector.tensor_scalar(outt[:], pimg[:], 1.0, None, OP.min)
                nc.sync.dma_start(out=out_d[:, t * PTILE:(t + 1) * PTILE], in_=outt[:])

    _legalize_waits(nc)
    res = run_bass_kernel_spmd(nc, in_maps, list(range(NCORES)))
    kernel.last_results = res

    for c in range(NCORES):
        o = res.results[c]["out"][:, :P_core].reshape(3, rows_pc, W_roi)
        out_img[0, :, y0 + c * rows_pc: y0 + (c + 1) * rows_pc, x0:x0 + W_roi] = o
    return out_img



# revision 13
# speedup vs baseline: 3.3305x; 3.3305x over previous
import math
import sys

import numpy as np

for _p in ("/opt/trn_rl_repo",):
    if _p not in sys.path:
        sys.path.insert(0, _p)

import ml_dtypes
from concourse import bass, mybir
from concourse.tile import TileContext
from concourse.bass_utils import run_bass_kernel_spmd

N = 4096
H = 384
W = 384
FOCAL = 0.5 * W / math.tan(0.5 * math.pi / 2.0)
CX, CY = W / 2.0, H / 2.0
CLIP_Z = 0.01
BLUR = 0.3
ALPHA_MIN = 1.0 / 255.0
NCORES = 8
GBLK = 128

f32 = mybir.dt.float32
f32r = mybir.dt.float32r
bf16 = mybir.dt.bfloat16
AF = mybir.ActivationFunctionType
OP = mybir.AluOpType
NP_BF16 = ml_dtypes.bfloat16


def _preprocess(xyz, scaling, opacity, rotation, features_dc):
    """Project gaussians (float64 on host), depth-sort, return per-gaussian
    screen params in front-to-back order."""
    xyz = xyz.astype(np.float64)
    x, y = xyz[:, 0], xyz[:, 1]
    z = xyz[:, 2] + 8.0
    zs = np.where(z > CLIP_Z, z, 1.0)

    scales = np.exp(scaling.astype(np.float64))
    q = rotation.astype(np.float64)
    q = q / np.linalg.norm(q, axis=-1, keepdims=True)
    w_, qx, qy, qz = q[:, 0], q[:, 1], q[:, 2], q[:, 3]
    R = np.empty((N, 3, 3), np.float64)
    R[:, 0, 0] = 1 - 2 * (qy * qy + qz * qz)
    R[:, 0, 1] = 2 * (qx * qy - w_ * qz)
    R[:, 0, 2] = 2 * (qx * qz + w_ * qy)
    R[:, 1, 0] = 2 * (qx * qy + w_ * qz)
    R[:, 1, 1] = 1 - 2 * (qx * qx + qz * qz)
    R[:, 1, 2] = 2 * (qy * qz - w_ * qx)
    R[:, 2, 0] = 2 * (qx * qz - w_ * qy)
    R[:, 2, 1] = 2 * (qy * qz + w_ * qx)
    R[:, 2, 2] = 1 - 2 * (qx * qx + qy * qy)
    M = R * scales[:, None, :]
    cov3d = np.einsum('nij,nkj->nik', M, M)

    tan_f = 0.5 * W / FOCAL
    tx = zs * np.clip(x / zs, -1.3 * tan_f, 1.3 * tan_f)
    ty = zs * np.clip(y / zs, -1.3 * tan_f, 1.3 * tan_f)
    rz, rz2 = 1.0 / zs, 1.0 / (zs * zs)
    J = np.zeros((N, 2, 3), np.float64)
    J[:, 0, 0] = FOCAL * rz
    J[:, 0, 2] = -FOCAL * tx * rz2
    J[:, 1, 1] = FOCAL * rz
    J[:, 1, 2] = -FOCAL * ty * rz2
    cov2d = np.einsum('nij,njk,nlk->nil', J, cov3d, J)
    c00 = cov2d[:, 0, 0] + BLUR
    c01 = cov2d[:, 0, 1]
    c11 = cov2d[:, 1, 1] + BLUR
    det = c00 * c11 - c01 * c01
    valid = (z > CLIP_Z) & (det > 0.0)
    det_s = np.where(valid, det, 1.0)
    conic = np.stack([c11, -c01, c00], -1) / det_s[:, None]

    cx = FOCAL * x * rz + CX
    cy = FOCAL * y * rz + CY
    rgbs = 1.0 / (1.0 + np.exp(-features_dc[:, 0, :].astype(np.float64)))
    opac = 1.0 / (1.0 + np.exp(-opacity[:, 0].astype(np.float64))) * valid

    order = np.argsort(np.where(valid, z, np.inf), kind='stable')
    return (conic[order], cx[order], cy[order], rgbs[order], opac[order],
            valid[order])


def _legalize_waits(nc):
    """The walrus codegen for compute-engine instruction structs accepts only
    one embedded sync wait. Move surplus waits onto same-engine NoOps placed
    immediately before the instruction."""
    skip = {"NoOp", "EventSemaphore", "Halt"}
    nid = [0]
    for blk in nc.main_func.blocks:
        out = []
        for inst in blk.instructions:
            si = getattr(inst, "sync_info", None)
            op = type(inst).__name__
            if (si is not None and si.on_wait and len(si.on_wait) > 1
                    and not any(s in op for s in skip)):
                waits = list(si.on_wait)
                for w in waits[:-1]:
                    nid[0] += 1
                    nop = mybir.InstNoOp(
                        name=f"{inst.name}-lw{nid[0]}", engine=inst.engine,
                        ins=[], outs=[],
                        sync_info=mybir.SyncInfo(on_wait=[w], on_update=[]))
                    out.append(nop)
                si.on_wait = [waits[-1]]
            out.append(inst)
        blk.instructions[:] = out


def _plan_tiles(ylo_g, yhi_g, live, y0, W_roi, P_all, pxmax):
    """Cut the flattened ROI pixel array into <= 16 tiles, balancing the
    per-tile gaussian-list sizes. Returns list of (start, end) pixel cuts."""
    yhi_sorted = np.sort(yhi_g[live])
    ylo_sorted = np.sort(ylo_g[live])
    nlive = len(yhi_sorted)

    def count(row_a, row_b):
        # gaussians whose [ylo, yhi] intersects [y0+row_a, y0+row_b]
        miss_hi = np.searchsorted(yhi_sorted, y0 + row_a, side='left')
        miss_lo = nlive - np.searchsorted(ylo_sorted, y0 + row_b, side='right')
        return nlive - miss_hi - miss_lo

    def greedy(ncap, pxcap):
        cuts = [0]
        while cuts[-1] < P_all and len(cuts) < 64:
            s = cuts[-1]
            rs = s // W_roi
            lo, hi = s + 1, min(s + pxcap, P_all)
            best = lo
            while lo <= hi:
                mid = (lo + hi) // 2
                if count(rs, (mid - 1) // W_roi) <= ncap:
                    best = mid
                    lo = mid + 1
                else:
                    hi = mid - 1
            cuts.append(best)
        return cuts

    best = None
    for ncap in range(512, 1600, 8):
        for pxcap in (256, 268, 284, 300, 320, 352, 384, 420, 460, pxmax):
            cuts = greedy(ncap, pxcap)
            nt = len(cuts) - 1
            if nt > 16:
                continue
            ns = [count(cuts[i] // W_roi, (cuts[i + 1] - 1) // W_roi)
                  for i in range(nt)]
            px = [cuts[i + 1] - cuts[i] for i in range(nt)]
            nbs = [(n + GBLK - 1) // GBLK for n in ns]
            while len(nbs) < 16:
                nbs.append(0)
                px.append(0)
            order = sorted(range(16), key=lambda i: (-nbs[i], px[i]))
            NB0 = max(max(nbs[i] for i in order[:8]), 1)
            NB1 = max(max(nbs[i] for i in order[8:]), 1)
            P0 = max(max(px[i] for i in order[:8]), 256)
            P1 = max(max(px[i] for i in order[8:]), 256)
            cost = NB0 * (P0 + 200) + NB1 * (P1 + 200)
            if best is None or cost < best[0]:
                best = (cost, cuts, order)
    return best[1], best[2]


def kernel(xyz, scaling, opacity, rotation, features_dc):
    conic, cx, cy, rgbs, opac, valid = _preprocess(
        xyz, scaling, opacity, rotation, features_dc)

    out_img = np.ones((1, 3, H, W), np.float32)
    A, B, C = conic[:, 0], conic[:, 1], conic[:, 2]
    with np.errstate(divide='ignore', invalid='ignore'):
        t_sig = np.log(np.maximum(opac, 1e-12) / ALPHA_MIN)
        det_c = C * A - B * B
        ry = np.sqrt(np.maximum(0.0, 2.0 * t_sig * A / np.maximum(det_c, 1e-12)))
        rx = np.sqrt(np.maximum(0.0, 2.0 * t_sig * C / np.maximum(det_c, 1e-12)))
    live = valid & (opac > ALPHA_MIN) & (t_sig > 0) & (det_c > 0)
    if not live.any():
        return out_img

    x0 = int(np.clip(np.floor((cx - rx)[live].min()), 0, W - 1))
    x1 = int(np.clip(np.ceil((cx + rx)[live].max()), 0, W - 1))
    y0 = int(np.clip(np.floor((cy - ry)[live].min()), 0, H - 1))
    y1 = int(np.clip(np.ceil((cy + ry)[live].max()), 0, H - 1))
    W_roi = x1 - x0 + 1
    H_roi = y1 - y0 + 1
    P_all = W_roi * H_roi

    ylo_g = cy - ry
    yhi_g = cy + ry
    cuts, order = _plan_tiles(ylo_g, yhi_g, live, y0, W_roi, P_all,
                              pxmax=512)
    NT = len(cuts) - 1
    T = 2  # slots per core (order maps 16 slots -> tiles; >16 tiles rejected)

    # per-tile gaussian lists (depth order preserved: arrays already sorted)
    tile_info = []
    for t in range(NT):
        s, e = cuts[t], cuts[t + 1]
        ra, rb = s // W_roi, (e - 1) // W_roi
        sel = np.nonzero(live & (yhi_g >= y0 + ra) & (ylo_g <= y0 + rb))[0]
        tile_info.append((s, e, sel))
    while len(tile_info) < 16:
        tile_info.append((0, 0, np.zeros(0, np.int64)))

    # slot shapes (shared across cores: SPMD)
    slot_tiles = [[order[s * NCORES + c] for c in range(NCORES)]
                  for s in range(T)]
    NBs, Ps = [], []
    for s in range(T):
        nb = max(max((len(tile_info[i][2]) + GBLK - 1) // GBLK
                     for i in slot_tiles[s]), 1)
        px = max(max(tile_info[i][1] - tile_info[i][0]
                     for i in slot_tiles[s]), 1)
        px = max(256, (px + 15) // 16 * 16)
        assert px <= 512
        NBs.append(nb)
        Ps.append(px)
    NBtot = sum(NBs)
    Ptot = sum(Ps)
    NBmax = max(NBs)

    # constant-blob layouts
    A_off = 0                      # c32 (float32r): a6 rows 0-5, NBtot*GBLK cols
    F_off = A_off + NBtot * GBLK   # feat: rows 0-5, Ptot cols
    C32 = F_off + Ptot
    S_off = GBLK                   # c16: [UTneg | -1 strip | UTe | ones | rgb]
    S_w = 2 * NBmax - 1
    U_off = S_off + S_w            # small UT+total col (strict upper): rows 0-6
    O_off = U_off + 8              # ones row: row 0, GBLK cols
    R_off = O_off + GBLK
    C16 = R_off + 3 * NBtot

    in_maps = []
    core_meta = []
    for c in range(NCORES):
        c32 = np.zeros((6, C32), np.float32)
        c16 = np.zeros((128, C16), np.float32)
        c16[:, 0:GBLK] = -np.triu(np.ones((GBLK, GBLK), np.float32), 1)
        c16[:, S_off + NBmax - 1] = -1.0
        c16[0:7, U_off:U_off + 8] = np.triu(np.ones((7, 8), np.float32), 1)
        c16[0, O_off:O_off + GBLK] = 1.0

        meta = []
        blk_base = 0
        p_base = 0
        for s in range(T):
            ti = slot_tiles[s][c]
            ps, pe, sel = tile_info[ti]
            n = len(sel)
            NB, P = NBs[s], Ps[s]
            # features (tile-centered coordinates)
            fx = np.full(P, 1e4, np.float64)
            fy = np.full(P, 1e4, np.float64)
            if pe > ps:
                pix = np.arange(ps, pe)
                px_x = (pix % W_roi) + x0
                px_y = (pix // W_roi) + y0
                xc, yc = px_x.mean(), px_y.mean()
                fx[:pe - ps] = px_x - xc
                fy[:pe - ps] = px_y - yc
            else:
                xc = yc = 0.0
            feat = np.stack([fx * fx, fy * fy, fx * fy, fx, fy,
                             np.ones(P)], 0)
            c32[0:6, F_off + p_base:F_off + p_base + P] = feat
            # gaussian coefficients
            a6 = np.zeros((6, NB * GBLK), np.float64)
            a6[5, :] = 1e4
            if n:
                gx = cx[sel] - xc
                gy = cy[sel] - yc
                c0, c1, c2 = A[sel], B[sel], C[sel]
                a6[0, :n] = 0.5 * c0
                a6[1, :n] = 0.5 * c2
                a6[2, :n] = c1
                a6[3, :n] = -(c0 * gx + c1 * gy)
                a6[4, :n] = -(c2 * gy + c1 * gx)
                a6[5, :n] = (0.5 * (c0 * gx * gx + c2 * gy * gy)
                             + c1 * gx * gy - np.log(opac[sel]))
                rgb = np.zeros((NB * GBLK, 3), np.float32)
                rgb[:n] = rgbs[sel]
                c16[:, R_off + 3 * blk_base:R_off + 3 * (blk_base + NB)] = \
                    rgb.reshape(NB, GBLK, 3).transpose(1, 0, 2).reshape(GBLK, 3 * NB)
            c32[0:6, A_off + blk_base * GBLK:A_off + (blk_base + NB) * GBLK] = a6
            meta.append((ps, pe))
            blk_base += NB
            p_base += P
        in_maps.append({"c32": c32,
                        "c16": c16.astype(NP_BF16)})
        core_meta.append(meta)

    nc = bass.Bass()
    c32_d = nc.declare_dram_parameter("c32", [6, C32], f32r, isOutput=False)
    c16_d = nc.declare_dram_parameter("c16", [128, C16], bf16, isOutput=False)
    out_d = nc.declare_dram_parameter("out", [3, Ptot], f32, isOutput=True)

    with TileContext(nc) as tc:
        with tc.tile_pool(name="const", bufs=1) as cp, \
             tc.tile_pool(name="work", bufs=3) as wp, \
             tc.tile_pool(name="ul", bufs=2 * max(NBs) + 2) as ulp, \
             tc.tile_pool(name="sb", bufs=2) as sbp, \
             tc.tile_pool(name="psig", bufs=1, space="PSUM") as sigp, \
             tc.tile_pool(name="pcum", bufs=1, space="PSUM") as cump, \
             tc.tile_pool(name="pball", bufs=2, space="PSUM") as ballp, \
             tc.tile_pool(name="pcarry", bufs=1, space="PSUM") as carryp, \
             tc.tile_pool(name="pimg", bufs=2, space="PSUM") as imgp:
            c32_sb = cp.tile([6, C32], f32r)
            c16_sb = cp.tile([128, C16], bf16)
            nc.sync.dma_start(out=c32_sb[:], in_=c32_d[:])
            nc.sync.dma_start(out=c16_sb[:], in_=c16_d[:])

            # warm-ups: absorb DMA waits once per engine; ACT warm-up also
            # loads the Exp table during the DMA window
            scr = cp.tile([1, 8], f32)
            pscr = sigp.tile([1, 1], f32, tag="sig")
            nc.scalar.activation(out=scr[0:1, 0:1], in_=c16_sb[0:1, 0:1],
                                 func=AF.Exp, scale=0.0)
            nc.vector.tensor_scalar_add(scr[0:1, 1:2], c16_sb[0:1, 0:1], 0.0)
            nc.tensor.matmul(pscr[:], c16_sb[0:1, 0:1], c16_sb[0:1, 0:1],
                             start=True, stop=True)

            utneg = c16_sb[:, 0:GBLK]
            ones_row = c16_sb[0:1, O_off:O_off + GBLK]

            u_tiles = [[None] * NBs[s] for s in range(T)]
            l1m_tiles = [[None] * NBs[s] for s in range(T)]
            pball_ps = [None] * T
            carry_sb = [None] * T
            pball_sb = [None] * T
            total_sb = [None] * T
            pimg_ps = [None] * T

            def a6_ap(s, b):
                base = sum(NBs[:s]) + b
                return c32_sb[0:6, A_off + base * GBLK:
                              A_off + (base + 1) * GBLK]

            def rgb_ap(s, b):
                base = sum(NBs[:s]) + b
                return c16_sb[:, R_off + 3 * base:R_off + 3 * (base + 1)]

            def feat_ap(s):
                base = sum(Ps[:s])
                return c32_sb[0:6, F_off + base:F_off + base + Ps[s]]

            # phase 1 (both slots): sigma -> alpha -> -ln(1-alpha) approx
            for s in range(T):
                NB, P = NBs[s], Ps[s]
                pball = ballp.tile([NB, P], f32, tag="ball")
                pball_ps[s] = pball
                for b in range(NB):
                    psig = sigp.tile([GBLK, P], f32, tag="sig")
                    nc.tensor.matmul(psig[:], a6_ap(s, b), feat_ap(s),
                                     start=True, stop=True)
                    u = ulp.tile([GBLK, P], bf16, tag="u")
                    nc.scalar.activation(out=u[:], in_=psig[:], func=AF.Exp,
                                         scale=-1.0)
                    u_tiles[s][b] = u
                    t1 = wp.tile([GBLK, P], bf16, tag="t1")
                    nc.vector.tensor_scalar(t1[:], u[:], 0.5, 1.0,
                                            OP.mult, OP.add)
                    l1m = ulp.tile([GBLK, P], bf16, tag="l1m")
                    nc.vector.tensor_tensor(l1m[:], t1[:], u[:], OP.mult)
                    l1m_tiles[s][b] = l1m
                    # -1 in window column b only: accumulates -sum(l1m_b)
                    # into pball row b (other rows get +0)
                    sel_ap = c16_sb[:, S_off + NBmax - 1 - b:
                                    S_off + NBmax - 1 - b + NB]
                    nc.tensor.matmul(pball[:, :], sel_ap, l1m[:],
                                     start=(b == 0), stop=(b == NB - 1))

            # phase 2 + 3 per slot
            for s in range(T):
                NB, P = NBs[s], Ps[s]
                pb_sb = sbp.tile([NB, P], bf16, tag=f"ballsb{s}")
                nc.vector.tensor_scalar_add(pb_sb[:], pball_ps[s][:], 0.0)
                pball_sb[s] = pb_sb
                # rows 0..NB-1: exclusive prefix (carry per block);
                # row NB: inclusive total (via the extra all-ones column)
                carry_ps = carryp.tile([NB + 1, P], f32, tag="carry")
                nc.tensor.matmul(carry_ps[:],
                                 c16_sb[0:NB, U_off:U_off + NB + 1],
                                 pb_sb[:], start=True, stop=True)
                ca_sb = sbp.tile([NB + 1, P], bf16, tag=f"carrymid{s}")
                nc.scalar.copy(ca_sb[:], carry_ps[:])
                # flatten carry rows onto partition 0 so they can serve as
                # matmul rhs operands (rhs base partition must be 0)
                ca_fl = sbp.tile([1, (NB + 1) * P], bf16, tag=f"carrysb{s}")
                nc.sync.dma_start(out=ca_fl[:], in_=ca_sb[:])
                carry_sb[s] = ca_fl

                pimg = imgp.tile([3, P], f32, tag="img")
                pimg_ps[s] = pimg
                for b in range(NB):
                    pcum = cump.tile([GBLK, P], f32, tag="cum")
                    nc.tensor.matmul(pcum[:], utneg, l1m_tiles[s][b][:],
                                     start=True, stop=False)
                    nc.tensor.matmul(pcum[:], ones_row,
                                     ca_fl[0:1, b * P:(b + 1) * P],
                                     start=False, stop=True)
                    tpre = wp.tile([GBLK, P], bf16, tag="tpre")
                    nc.scalar.activation(out=tpre[:], in_=pcum[:], func=AF.Exp)
                    w_t = wp.tile([GBLK, P], bf16, tag="w")
                    nc.vector.tensor_tensor(w_t[:], tpre[:], u_tiles[s][b][:],
                                            OP.mult)
                    nc.tensor.matmul(pimg[:], rgb_ap(s, b), w_t[:],
                                     start=(b == 0), stop=False)

            # background + clamp + output
            for s in range(T):
                NB, P = NBs[s], Ps[s]
                tfin = sbp.tile([1, P], bf16, tag=f"tfin{s}")
                nc.scalar.activation(out=tfin[:],
                                     in_=carry_sb[s][0:1, NB * P:(NB + 1) * P],
                                     func=AF.Exp)
                nc.tensor.matmul(pimg_ps[s][:],
                                 c16_sb[0:1, O_off:O_off + 3],
                                 tfin[:], start=False, stop=True)
                outt = sbp.tile([3, P], f32, tag=f"outt{s}")
                nc.vector.tensor_scalar(outt[:], pimg_ps[s][:], 1.0, None,
                                        OP.min)
                base = sum(Ps[:s])
                nc.sync.dma_start(out=out_d[:, base:base + P], in_=outt[:])

    _legalize_waits(nc)
    res = run_bass_kernel_spmd(nc, in_maps, list(range(NCORES)))
    kernel.last_results = res

    canvas = np.ones((P_all, 3), np.float32)
    for c in range(NCORES):
        o = res.results[c]["out"]
        p_base = 0
        for s in range(T):
            ps, pe = core_meta[c][s]
            if pe > ps:
                canvas[ps:pe] = o[:, p_base:p_base + (pe - ps)].T
            p_base += Ps[s]
    out_img[0, :, y0:y0 + H_roi, x0:x0 + W_roi] = \
        canvas.reshape(H_roi, W_roi, 3).transpose(2, 0, 1)
    return out_img


# revision 20
# speedup vs baseline: 5.0134x; 1.5053x over previous
import math
import sys

import numpy as np

for _p in ("/opt/trn_rl_repo",):
    if _p not in sys.path:
        sys.path.insert(0, _p)

import ml_dtypes
from concourse import bass, mybir
from concourse.tile import TileContext
from concourse.bass_utils import run_bass_kernel_spmd

N = 4096
H = 384
W = 384
FOCAL = 0.5 * W / math.tan(0.5 * math.pi / 2.0)
CX, CY = W / 2.0, H / 2.0
CLIP_Z = 0.01
BLUR = 0.3
ALPHA_MIN = 1.0 / 255.0
NCORES = 8
GBLK = 128   # partitions per block
GEFF = 127   # gaussians per block (partition 0 carries transmittance)

f32 = mybir.dt.float32
bf16 = mybir.dt.bfloat16
AF = mybir.ActivationFunctionType
OP = mybir.AluOpType
NP_BF16 = ml_dtypes.bfloat16


def _preprocess(xyz, scaling, opacity, rotation, features_dc):
    """Project gaussians (float64 on host), depth-sort, return per-gaussian
    screen params in front-to-back order."""
    xyz = xyz.astype(np.float64)
    x, y = xyz[:, 0], xyz[:, 1]
    z = xyz[:, 2] + 8.0
    zs = np.where(z > CLIP_Z, z, 1.0)

    scales = np.exp(scaling.astype(np.float64))
    q = rotation.astype(np.float64)
    q = q / np.linalg.norm(q, axis=-1, keepdims=True)
    w_, qx, qy, qz = q[:, 0], q[:, 1], q[:, 2], q[:, 3]
    R = np.empty((N, 3, 3), np.float64)
    R[:, 0, 0] = 1 - 2 * (qy * qy + qz * qz)
    R[:, 0, 1] = 2 * (qx * qy - w_ * qz)
    R[:, 0, 2] = 2 * (qx * qz + w_ * qy)
    R[:, 1, 0] = 2 * (qx * qy + w_ * qz)
    R[:, 1, 1] = 1 - 2 * (qx * qx + qz * qz)
    R[:, 1, 2] = 2 * (qy * qz - w_ * qx)
    R[:, 2, 0] = 2 * (qx * qz - w_ * qy)
    R[:, 2, 1] = 2 * (qy * qz + w_ * qx)
    R[:, 2, 2] = 1 - 2 * (qx * qx + qy * qy)
    M = R * scales[:, None, :]
    cov3d = np.einsum('nij,nkj->nik', M, M)

    tan_f = 0.5 * W / FOCAL
    tx = zs * np.clip(x / zs, -1.3 * tan_f, 1.3 * tan_f)
    ty = zs * np.clip(y / zs, -1.3 * tan_f, 1.3 * tan_f)
    rz, rz2 = 1.0 / zs, 1.0 / (zs * zs)
    J = np.zeros((N, 2, 3), np.float64)
    J[:, 0, 0] = FOCAL * rz
    J[:, 0, 2] = -FOCAL * tx * rz2
    J[:, 1, 1] = FOCAL * rz
    J[:, 1, 2] = -FOCAL * ty * rz2
    cov2d = np.einsum('nij,njk,nlk->nil', J, cov3d, J)
    c00 = cov2d[:, 0, 0] + BLUR
    c01 = cov2d[:, 0, 1]
    c11 = cov2d[:, 1, 1] + BLUR
    det = c00 * c11 - c01 * c01
    valid = (z > CLIP_Z) & (det > 0.0)
    det_s = np.where(valid, det, 1.0)
    conic = np.stack([c11, -c01, c00], -1) / det_s[:, None]

    cx = FOCAL * x * rz + CX
    cy = FOCAL * y * rz + CY
    rgbs = 1.0 / (1.0 + np.exp(-features_dc[:, 0, :].astype(np.float64)))
    opac = 1.0 / (1.0 + np.exp(-opacity[:, 0].astype(np.float64))) * valid

    order = np.argsort(np.where(valid, z, np.inf), kind='stable')
    return (conic[order], cx[order], cy[order], rgbs[order], opac[order],
            valid[order])


def _legalize_waits(nc):
    """The walrus codegen for compute-engine instruction structs accepts only
    one embedded sync wait. Move surplus waits onto same-engine NoOps placed
    immediately before the instruction."""
    skip = {"NoOp", "EventSemaphore", "Halt"}
    nid = [0]
    for blk in nc.main_func.blocks:
        out = []
        for inst in blk.instructions:
            si = getattr(inst, "sync_info", None)
            op = type(inst).__name__
            if (si is not None and si.on_wait and len(si.on_wait) > 1
                    and not any(s in op for s in skip)):
                waits = list(si.on_wait)
                for w in waits[:-1]:
                    nid[0] += 1
                    nop = mybir.InstNoOp(
                        name=f"{inst.name}-lw{nid[0]}", engine=inst.engine,
                        ins=[], outs=[],
                        sync_info=mybir.SyncInfo(on_wait=[w], on_update=[]))
                    out.append(nop)
                si.on_wait = [waits[-1]]
            out.append(inst)
        blk.instructions[:] = out


def _hilo(x):
    """Split fp32 array into bf16 hi/lo pair (x ~= hi + lo)."""
    x = x.astype(np.float32)
    hi = x.astype(NP_BF16).astype(np.float32)
    lo = (x - hi).astype(NP_BF16).astype(np.float32)
    return hi, lo


def _plan_tiles(ylo_g, yhi_g, live, y0, W_roi, P_all, pxmax):
    """Cut the flattened ROI pixel array into <= 16 tiles, balancing the
    per-tile gaussian-list sizes. Returns (cuts, order)."""
    yhi_sorted = np.sort(yhi_g[live])
    ylo_sorted = np.sort(ylo_g[live])
    nlive = len(yhi_sorted)

    def count(row_a, row_b):
        miss_hi = np.searchsorted(yhi_sorted, y0 + row_a, side='left')
        miss_lo = nlive - np.searchsorted(ylo_sorted, y0 + row_b, side='right')
        return nlive - miss_hi - miss_lo

    def greedy(ncap, pxcap):
        cuts = [0]
        while cuts[-1] < P_all and len(cuts) < 64:
            s = cuts[-1]
            rs = s // W_roi
            lo, hi = s + 1, min(s + pxcap, P_all)
            best = lo
            while lo <= hi:
                mid = (lo + hi) // 2
                if count(rs, (mid - 1) // W_roi) <= ncap:
                    best = mid
                    lo = mid + 1
                else:
                    hi = mid - 1
            cuts.append(best)
        return cuts

    best = None
    for ncap in range(508, 1600, 8):
        for pxcap in (256, 268, 284, 300, 320, 352, 384, 420, 460, pxmax):
            cuts = greedy(ncap, pxcap)
            nt = len(cuts) - 1
            if nt > 16:
                continue
            ns = [count(cuts[i] // W_roi, (cuts[i + 1] - 1) // W_roi)
                  for i in range(nt)]
            px = [cuts[i + 1] - cuts[i] for i in range(nt)]
            nbs = [(n + GEFF - 1) // GEFF for n in ns]
            while len(nbs) < 16:
                nbs.append(0)
                px.append(0)
            order = sorted(range(16), key=lambda i: (-nbs[i], px[i]))
            NB0 = max(max(nbs[i] for i in order[:8]), 1)
            NB1 = max(max(nbs[i] for i in order[8:]), 1)
            P0 = max(max(px[i] for i in order[:8]), 256)
            P1 = max(max(px[i] for i in order[8:]), 256)
            cost = NB0 * (P0 + 200) + NB1 * (P1 + 200)
            if best is None or cost < best[0]:
                best = (cost, cuts, order)
    return best[1], best[2]


def kernel(xyz, scaling, opacity, rotation, features_dc):
    conic, cx, cy, rgbs, opac, valid = _preprocess(
        xyz, scaling, opacity, rotation, features_dc)

    out_img = np.ones((1, 3, H, W), np.float32)
    A, B, C = conic[:, 0], conic[:, 1], conic[:, 2]
    with np.errstate(divide='ignore', invalid='ignore'):
        t_sig = np.log(np.maximum(opac, 1e-12) / ALPHA_MIN)
        det_c = C * A - B * B
        ry = np.sqrt(np.maximum(0.0, 2.0 * t_sig * A / np.maximum(det_c, 1e-12)))
        rx = np.sqrt(np.maximum(0.0, 2.0 * t_sig * C / np.maximum(det_c, 1e-12)))
    live = valid & (opac > ALPHA_MIN) & (t_sig > 0) & (det_c > 0)
    if not live.any():
        return out_img

    x0 = int(np.clip(np.floor((cx - rx)[live].min()), 0, W - 1))
    x1 = int(np.clip(np.ceil((cx + rx)[live].max()), 0, W - 1))
    y0 = int(np.clip(np.floor((cy - ry)[live].min()), 0, H - 1))
    y1 = int(np.clip(np.ceil((cy + ry)[live].max()), 0, H - 1))
    W_roi = x1 - x0 + 1
    H_roi = y1 - y0 + 1
    P_all = W_roi * H_roi

    ylo_g = cy - ry
    yhi_g = cy + ry
    cuts, order = _plan_tiles(ylo_g, yhi_g, live, y0, W_roi, P_all, pxmax=512)
    NT = len(cuts) - 1
    T = 2

    tile_info = []
    for t in range(NT):
        s, e = cuts[t], cuts[t + 1]
        ra, rb = s // W_roi, (e - 1) // W_roi
        sel = np.nonzero(live & (yhi_g >= y0 + ra) & (ylo_g <= y0 + rb))[0]
        tile_info.append((s, e, sel))
    while len(tile_info) < 16:
        tile_info.append((0, 0, np.zeros(0, np.int64)))

    slot_tiles = [[order[s * NCORES + c] for c in range(NCORES)]
                  for s in range(T)]
    NBs, Ps = [], []
    for s in range(T):
        nb = max(max((len(tile_info[i][2]) + GEFF - 1) // GEFF
                     for i in slot_tiles[s]), 1)
        px = max(max(tile_info[i][1] - tile_info[i][0]
                     for i in slot_tiles[s]), 1)
        px = max(256, (px + 15) // 16 * 16)
        assert px <= 512
        NBs.append(nb)
        Ps.append(px)
    NBtot = sum(NBs)
    Ptot = sum(Ps)

    # c16 blob layouts (all bf16):
    #  cA [128, .]: UTc matrix | ones3 | rgb per block
    #  cB [18, .]: a6 hi/lo (18 rows) | feat hi/lo (18 rows)
    O_off = GBLK                   # ones row (row 0); first 3 cols = ones3
    R_off = O_off + GBLK
    CA = R_off + 3 * NBtot
    F_off = NBtot * GBLK
    CB = F_off + Ptot

    # UTc: col j accumulates -sum(l1m[1<=g<j]) plus the carry (partition 0
    # holds the running log-transmittance); col 0 yields the next carry =
    # carry - sum(all real l1m)
    utc = -np.triu(np.ones((GBLK, GBLK), np.float32), 1)
    utc[0, :] = 1.0
    utc[1:, 0] = -1.0

    in_maps = []
    core_meta = []
    for c in range(NCORES):
        cA = np.zeros((GBLK, CA), np.float32)
        cB = np.zeros((18, CB), np.float32)
        cA[:, 0:GBLK] = utc
        cA[0, O_off:O_off + GBLK] = 1.0

        meta = []
        blk_base = 0
        p_base = 0
        for s in range(T):
            ti = slot_tiles[s][c]
            ps, pe, sel = tile_info[ti]
            n = len(sel)
            NB, P = NBs[s], Ps[s]
            fx = np.full(P, 1e4, np.float64)
            fy = np.full(P, 1e4, np.float64)
            if pe > ps:
                pix = np.arange(ps, pe)
                px_x = (pix % W_roi) + x0
                px_y = (pix // W_roi) + y0
                xc = round(float(px_x.mean()))
                yc = round(float(px_y.mean()))
                fx[:pe - ps] = px_x - xc
                fy[:pe - ps] = px_y - yc
            else:
                xc = yc = 0.0
            feat = np.stack([fx * fx, fy * fy, fx * fy, fx, fy,
                             np.ones(P)], 0).astype(np.float32)
            fh, fl = _hilo(feat)
            cB[0:6, F_off + p_base:F_off + p_base + P] = fh
            cB[6:12, F_off + p_base:F_off + p_base + P] = fl
            cB[12:18, F_off + p_base:F_off + p_base + P] = fh

            a6 = np.zeros((6, NB * GBLK), np.float64)
            a6[5, :] = 1e4
            if n:
                gx = cx[sel] - xc
                gy = cy[sel] - yc
                c0, c1, c2 = A[sel], B[sel], C[sel]
                # block b holds gaussians [b*GEFF, (b+1)*GEFF) in partitions
                # 1..127; partition 0 is the carry channel (dummy gaussian)
                gi = np.arange(n)
                col = (gi // GEFF) * GBLK + (gi % GEFF) + 1
                a6[0, col] = 0.5 * c0
                a6[1, col] = 0.5 * c2
                a6[2, col] = c1
                a6[3, col] = -(c0 * gx + c1 * gy)
                a6[4, col] = -(c2 * gy + c1 * gx)
                a6[5, col] = (0.5 * (c0 * gx * gx + c2 * gy * gy)
                              + c1 * gx * gy - np.log(opac[sel]))
                rgb = np.zeros((NB * GBLK, 3), np.float32)
                rgb[col] = rgbs[sel]
                cA[:, R_off + 3 * blk_base:R_off + 3 * (blk_base + NB)] = \
                    rgb.reshape(NB, GBLK, 3).transpose(1, 0, 2).reshape(GBLK, 3 * NB)
            ah, al = _hilo(a6.astype(np.float32))
            a_sl = slice(blk_base * GBLK, (blk_base + NB) * GBLK)
            # pairs with feat rows [fh; fl; fh]: ah*fh + ah*fl + al*fh
            cB[0:6, a_sl] = ah
            cB[6:12, a_sl] = ah
            cB[12:18, a_sl] = al
            meta.append((ps, pe))
            blk_base += NB
            p_base += P
        in_maps.append({"cA": cA.astype(NP_BF16), "cB": cB.astype(NP_BF16)})
        core_meta.append(meta)

    nc = bass.Bass()
    cA_d = nc.declare_dram_parameter("cA", [GBLK, CA], bf16, isOutput=False)
    cB_d = nc.declare_dram_parameter("cB", [18, CB], bf16, isOutput=False)
    out_d = nc.declare_dram_parameter("out", [3, Ptot], f32, isOutput=True)

    with TileContext(nc) as tc:
        with tc.tile_pool(name="const", bufs=1) as cp, \
             tc.tile_pool(name="work", bufs=6) as wp, \
             tc.tile_pool(name="ul", bufs=6) as ulp, \
             tc.tile_pool(name="sb", bufs=2) as sbp, \
             tc.tile_pool(name="psig", bufs=2, space="PSUM") as sigp, \
             tc.tile_pool(name="pcum", bufs=3, space="PSUM") as cump, \
             tc.tile_pool(name="pimg", bufs=1, space="PSUM") as imgp:
            cA_sb = cp.tile([GBLK, CA], bf16)
            cB_sb = cp.tile([18, CB], bf16)
            nc.sync.dma_start(out=cA_sb[:], in_=cA_d[:])
            nc.sync.dma_start(out=cB_sb[:], in_=cB_d[:])

            # warm-ups: absorb DMA waits; ACT warm-up loads the Exp table
            scr = cp.tile([1, 8], f32)
            pscr = sigp.tile([1, 1], f32, tag="sig")
            nc.scalar.activation(out=scr[0:1, 0:1], in_=cA_sb[0:1, 0:1],
                                 func=AF.Exp, scale=0.0)
            nc.vector.tensor_scalar_add(scr[0:1, 1:2], cA_sb[0:1, 0:1], 0.0)
            nc.gpsimd.tensor_scalar_add(scr[0:1, 2:3], cA_sb[0:1, 0:1], 0.0)
            nc.tensor.matmul(pscr[:], cA_sb[0:1, 0:1], cA_sb[0:1, 0:1],
                             start=True, stop=True)

            utc_ap = cA_sb[:, 0:GBLK]
            ones_row = cA_sb[0:1, O_off:O_off + GBLK]
            ones3 = cA_sb[0:1, O_off:O_off + 3]

            def a6_ap(s, b):
                base = sum(NBs[:s]) + b
                return cB_sb[:, base * GBLK:(base + 1) * GBLK]

            def rgb_ap(s, b):
                base = sum(NBs[:s]) + b
                return cA_sb[:, R_off + 3 * base:R_off + 3 * (base + 1)]

            def feat_ap(s):
                base = sum(Ps[:s])
                return cB_sb[:, F_off + base:F_off + base + Ps[s]]

            u_t = [[None] * NBs[s] for s in range(T)]
            l1m_t = [[None] * NBs[s] for s in range(T)]
            pcum_t = [[None] * NBs[s] for s in range(T)]
            carry_t = [None] * T
            pimg_ps = [None] * T

            def prefix(s, b):
                P = Ps[s]
                psig = sigp.tile([GBLK, P], f32, tag="sig")
                nc.tensor.matmul(psig[:], a6_ap(s, b), feat_ap(s),
                                 start=True, stop=True)
                u = ulp.tile([GBLK, P], bf16, tag="u")
                nc.scalar.activation(out=u[:], in_=psig[:], func=AF.Exp,
                                     scale=-1.0)
                u_t[s][b] = u
                t1 = wp.tile([GBLK, P], bf16, tag="t1")
                nc.vector.tensor_scalar(t1[:], u[:], 0.5, 1.0, OP.mult, OP.add)
                l1m = ulp.tile([GBLK, P], bf16, tag="l1m")
                nc.vector.tensor_tensor(l1m[:], t1[:], u[:], OP.mult)
                l1m_t[s][b] = l1m

            def tail(s, b):
                P = Ps[s]
                NB = NBs[s]
                pcum = cump.tile([GBLK, P], f32, tag="cum")
                nc.tensor.matmul(pcum[:], utc_ap, l1m_t[s][b][:],
                                 start=True, stop=(b == 0))
                if b > 0:
                    # broadcast the running log-transmittance (carry) onto
                    # all partitions; pcum[0] then holds the next carry
                    nc.tensor.matmul(pcum[:], ones_row, carry_t[s][:],
                                     start=False, stop=True)
                pcum_t[s][b] = pcum
                if b + 1 < NB:
                    cnext = wp.tile([1, P], bf16, tag="carry", name="cnext")
                    nc.vector.tensor_scalar_add(cnext[:], pcum[0:1, :], 0.0)
                    carry_t[s] = cnext
                tpre = wp.tile([GBLK, P], bf16, tag="tpre")
                nc.scalar.activation(out=tpre[:], in_=pcum[:], func=AF.Exp)
                w_tl = wp.tile([GBLK, P], bf16, tag="w")
                nc.vector.tensor_tensor(w_tl[:], tpre[:], u_t[s][b][:],
                                        OP.mult)
                nc.tensor.matmul(pimg_ps[s][:], rgb_ap(s, b), w_tl[:],
                                 start=(b == 0), stop=False)

            for s in range(T):
                pimg_ps[s] = imgp.tile([3, Ps[s]], f32, tag=f"img{s}",
                                       name=f"pimg{s}")
                prefix(s, 0)
            for r in range(max(NBs)):
                for s in range(T):
                    if r + 1 < NBs[s]:
                        prefix(s, r + 1)
                    if r < NBs[s]:
                        tail(s, r)

            for s in range(T):
                NB, P = NBs[s], Ps[s]
                tfin = sbp.tile([1, P], bf16, tag=f"tfin{s}")
                nc.scalar.activation(out=tfin[:],
                                     in_=pcum_t[s][NB - 1][0:1, :],
                                     func=AF.Exp)
                nc.tensor.matmul(pimg_ps[s][:], ones3, tfin[:],
                                 start=False, stop=True)
                outt = sbp.tile([3, P], f32, tag=f"outt{s}")
                nc.vector.tensor_scalar(outt[:], pimg_ps[s][:], 1.0, None,
                                        OP.min)
                base = sum(Ps[:s])
                nc.sync.dma_start(out=out_d[:, base:base + P], in_=outt[:])

    _legalize_waits(nc)
    res = run_bass_kernel_spmd(nc, in_maps, list(range(NCORES)))
    kernel.last_results = res

    canvas = np.ones((P_all, 3), np.float32)
    for c in range(NCORES):
        o = res.results[c]["out"]
        p_base = 0
        for s in range(T):
            ps, pe = core_meta[c][s]
            if pe > ps:
                canvas[ps:pe] = o[:, p_base:p_base + (pe - ps)].T
            p_base += Ps[s]
    out_img[0, :, y0:y0 + H_roi, x0:x0 + W_roi] = \
        canvas.reshape(H_roi, W_roi, 3).transpose(2, 0, 1)
    return out_img
